# revision 3
# baseline (speedup 1.0000x reference)
"""Multi-head attention (B=4, S=2048, D=1024, H=16) on 8 Trainium2 NeuronCores.

Sharding: core c handles batch b = c//2 and query-row half c%2 (1024 query
rows). Each core computes K/V projections for its batch's full 2048 kv rows
(duplicated across the pair), attention for its 1024 query rows over all 16
heads, and the output projection for its rows. Output is a pure concatenation
across cores — no collectives.

Device algorithm (per core, all matmuls bf16 with fp32 PSUM accumulation):
  QT[e,s]  = (WqT.T @ xqT) * 1/8 + bq/8          (e on partitions)
  KT[e,s]  =  WkT.T @ xkT                        (bk dropped: softmax-invariant)
  V[s,e]   =  xvT.T @ WvT                        (bv folded into output bias)
  per head h, q-chunk:
    scoresT[k,q] = KT_h.T @ QT_h                 (K=64 contraction; head parity
                                                  lands on partition halves ->
                                                  row-group-packed matmuls)
    E = exp(scoresT)            on ScalarE, PSUM->SBUF bf16
    [y_h; rowsum] = [V_h | 1].T @ E              (M=65, ones col gives rowsum)
    yT_h = y_h * (1/rowsum)     (PE K=1 matmul replicates 1/rowsum across
                                 partitions; DVE multiply)
  out[q,e] = yT.T @ WoT + (bo + Wo@bv)
"""

import numpy as np
import ml_dtypes

B, S, D = 4, 2048, 1024
H, DK = 16, 64
NCORES = 8
SQ = S // 2            # query rows per core
P = 128
DTILES = D // P        # 8 contraction tiles
QCH = SQ // 512        # 2 q chunks of 512
KT_N = S // P          # 16 kv 128-tiles
HP = H // 2            # 8 head pairs
CHUNK = 512

_CACHE = {}


def _build_nc():
    import concourse.mybir as mybir
    import concourse.tile as tile
    from concourse import bacc

    F32, BF16 = mybir.dt.float32, mybir.dt.bfloat16
    Copy = mybir.ActivationFunctionType.Copy
    Exp = mybir.ActivationFunctionType.Exp

    nc = bacc.Bacc("TRN2", target_bir_lowering=False, debug=False,
                   num_devices=NCORES)

    xqT = nc.dram_tensor("xqT", [D, SQ], BF16, kind="ExternalInput").ap()
    xkT = nc.dram_tensor("xkT", [D, S], BF16, kind="ExternalInput").ap()
    xvT = nc.dram_tensor("xvT", [D, S], BF16, kind="ExternalInput").ap()
    wqT = nc.dram_tensor("wqT", [D, D], BF16, kind="ExternalInput").ap()
    wkT = nc.dram_tensor("wkT", [D, D], BF16, kind="ExternalInput").ap()
    wvT = nc.dram_tensor("wvT", [D, D], BF16, kind="ExternalInput").ap()
    woT = nc.dram_tensor("woT", [D, D], BF16, kind="ExternalInput").ap()
    bqs = nc.dram_tensor("bqs", [P, DTILES], F32, kind="ExternalInput").ap()
    bob = nc.dram_tensor("bob", [P, D], F32, kind="ExternalInput").ap()
    out = nc.dram_tensor("out", [SQ, D], F32, kind="ExternalOutput").ap()

    def pdt(ap):  # [D, N] dram -> [P, DTILES, N] partition-tiled view
        return ap.rearrange("(a p) n -> p a n", p=P)

    with tile.TileContext(nc) as tc:
        with (
            tc.tile_pool(name="wpool", bufs=2) as wpool,
            tc.tile_pool(name="xpool", bufs=2) as xpool,
            tc.tile_pool(name="cpool", bufs=1) as cpool,
            tc.tile_pool(name="epool", bufs=5) as epool,
            tc.tile_pool(name="ytpool", bufs=2) as ytpool,
            tc.tile_pool(name="npool", bufs=2) as npool,
            tc.tile_pool(name="opool", bufs=2) as opool,
            tc.tile_pool(name="psP", bufs=2, space="PSUM") as psP,
            tc.tile_pool(name="psS", bufs=4, space="PSUM") as psS,
            tc.tile_pool(name="psA", bufs=2, space="PSUM") as psA,
        ):
            # ---- constants / residents ----
            wq_s = wpool.tile([P, DTILES, D], BF16, tag="w", name="wq_s")
            nc.sync.dma_start(wq_s[:], pdt(wqT))
            bq_s = cpool.tile([P, DTILES], F32, name="bq_s")
            nc.sync.dma_start(bq_s[:], bqs[:])
            bob_s = cpool.tile([P, D], F32, name="bob_s")
            nc.sync.dma_start(bob_s[:], bob[:])
            ones_s = cpool.tile([1, DK], BF16, name="ones_s")
            nc.gpsimd.memset(ones_s[:], 1.0)

            qt_s = cpool.tile([P, DTILES, SQ], BF16, name="qt_s")
            kt_s = cpool.tile([P, DTILES, S], BF16, name="kt_s")
            va_s = cpool.tile([P, KT_N, H * (DK + 1)], BF16, name="va_s")
            nc.gpsimd.memset(va_s[:], 1.0)  # ones cols survive the V copies

            # ---- Q projection: QT = WqT.T @ xqT, scaled by 1/8, +bq/8 ----
            for qc in range(QCH):
                xq_c = xpool.tile([P, DTILES, CHUNK], BF16, tag="x", name="xq_c")
                nc.sync.dma_start(
                    xq_c[:], pdt(xqT[:, qc * CHUNK:(qc + 1) * CHUNK]))
                for et in range(DTILES):
                    psq = psP.tile([P, CHUNK], F32, tag="p", name="psq")
                    for dt in range(DTILES):
                        nc.tensor.matmul(
                            psq[:],
                            wq_s[:, dt, et * P:(et + 1) * P],
                            xq_c[:, dt, :],
                            start=(dt == 0), stop=(dt == DTILES - 1))
                    nc.vector.tensor_scalar(
                        qt_s[:, et, qc * CHUNK:(qc + 1) * CHUNK], psq[:],
                        0.125, bq_s[:, et:et + 1],
                        mybir.AluOpType.mult, mybir.AluOpType.add)

            # ---- K projection: KT = WkT.T @ xkT ----
            wk_s = wpool.tile([P, DTILES, D], BF16, tag="w", name="wk_s")
            nc.sync.dma_start(wk_s[:], pdt(wkT))
            for sc in range(S // CHUNK):
                xk_c = xpool.tile([P, DTILES, CHUNK], BF16, tag="x", name="xk_c")
                nc.sync.dma_start(
                    xk_c[:], pdt(xkT[:, sc * CHUNK:(sc + 1) * CHUNK]))
                for et in range(DTILES):
                    psk = psP.tile([P, CHUNK], F32, tag="p", name="psk")
                    for dt in range(DTILES):
                        nc.tensor.matmul(
                            psk[:],
                            wk_s[:, dt, et * P:(et + 1) * P],
                            xk_c[:, dt, :],
                            start=(dt == 0), stop=(dt == DTILES - 1))
                    nc.vector.tensor_copy(
                        kt_s[:, et, sc * CHUNK:(sc + 1) * CHUNK], psk[:])

            # ---- V projection: V = xvT.T @ WvT (no bias; folded into bob) ----
            wv_s = wpool.tile([P, DTILES, D], BF16, tag="w", name="wv_s")
            nc.sync.dma_start(wv_s[:], pdt(wvT))
            for sc in range(S // CHUNK):
                xv_c = xpool.tile([P, DTILES, CHUNK], BF16, tag="x", name="xv_c")
                nc.sync.dma_start(
                    xv_c[:], pdt(xvT[:, sc * CHUNK:(sc + 1) * CHUNK]))
                for stl in range(CHUNK // P):
                    st = sc * (CHUNK // P) + stl
                    for ec in range(D // CHUNK):
                        psv = psP.tile([P, CHUNK], F32, tag="p", name="psv")
                        for dt in range(DTILES):
                            nc.tensor.matmul(
                                psv[:],
                                xv_c[:, dt, stl * P:(stl + 1) * P],
                                wv_s[:, dt, ec * CHUNK:(ec + 1) * CHUNK],
                                start=(dt == 0), stop=(dt == DTILES - 1))
                        for hl in range(CHUNK // DK):
                            h = ec * (CHUNK // DK) + hl
                            nc.vector.tensor_copy(
                                va_s[:, st, h * (DK + 1):h * (DK + 1) + DK],
                                psv[:, hl * DK:(hl + 1) * DK])

            # ---- load WoT while attention runs ----
            wo_s = wpool.tile([P, DTILES, D], BF16, tag="w", name="wo_s")
            nc.sync.dma_start(wo_s[:], pdt(woT))

            # ---- attention + output projection, per q chunk ----
            for qc in range(QCH):
                qsl = slice(qc * CHUNK, (qc + 1) * CHUNK)
                yt_c = ytpool.tile([P, DTILES, CHUNK], BF16, tag="yt",
                                   name="yt_c")
                for hp in range(HP):
                    # scores + exp, head parity interleaved (row-group packing)
                    ex = {}
                    for par in (0, 1):
                        for half in (0, 1):
                            ex[par, half] = epool.tile(
                                [P, KT_N // 2, CHUNK], BF16, tag="e",
                                name="ex")
                    for kt in range(KT_N):
                        half, kh = kt // 8, kt % 8
                        for par in (0, 1):
                            pb = DK * par
                            pss = psS.tile([P, CHUNK], F32, tag="s",
                                           name="pss")
                            nc.tensor.matmul(
                                pss[:],
                                kt_s[pb:pb + DK, hp, kt * P:(kt + 1) * P],
                                qt_s[pb:pb + DK, hp, qsl],
                                start=True, stop=True)
                            nc.scalar.activation(
                                ex[par, half][:, kh, :], pss[:], Exp)
                    # attn @ [V | 1], then normalize
                    for par in (0, 1):
                        h = 2 * hp + par
                        psa = psA.tile([DK + 1, CHUNK], F32, tag="a",
                                       name="psa")
                        for kt in range(KT_N):
                            nc.tensor.matmul(
                                psa[:],
                                va_s[:, kt, h * (DK + 1):(h + 1) * (DK + 1)],
                                ex[par, kt // 8][:, kt % 8, :],
                                start=(kt == 0), stop=(kt == KT_N - 1))
                        rs = npool.tile([1, CHUNK], F32, tag="rs", name="rs")
                        nc.vector.reciprocal(rs[:], psa[DK:DK + 1, :])
                        rsb = npool.tile([1, CHUNK], BF16, tag="rsb",
                                         name="rsb")
                        nc.vector.tensor_copy(rsb[:], rs[:])
                        psr = psP.tile([DK, CHUNK], F32, tag="p", name="psr")
                        nc.tensor.matmul(psr[:], ones_s[:], rsb[:],
                                         start=True, stop=True)
                        rbc = npool.tile([DK, CHUNK], F32, tag="rbc",
                                         name="rbc")
                        nc.vector.tensor_copy(rbc[:], psr[:])
                        nc.vector.tensor_mul(
                            yt_c[DK * par:DK * (par + 1), hp, :],
                            psa[0:DK, :], rbc[:])

                # output projection for this q chunk
                for qtl in range(CHUNK // P):
                    for ec in range(D // CHUNK):
                        psf = psP.tile([P, CHUNK], F32, tag="p", name="psf")
                        for j in range(DTILES):
                            nc.tensor.matmul(
                                psf[:],
                                yt_c[:, j, qtl * P:(qtl + 1) * P],
                                wo_s[:, j, ec * CHUNK:(ec + 1) * CHUNK],
                                start=(j == 0), stop=(j == DTILES - 1))
                        osb = opool.tile([P, CHUNK], F32, tag="o", name="osb")
                        nc.vector.tensor_add(
                            osb[:], psf[:],
                            bob_s[:, ec * CHUNK:(ec + 1) * CHUNK])
                        r0 = qc * CHUNK + qtl * P
                        nc.sync.dma_start(
                            out[r0:r0 + P, ec * CHUNK:(ec + 1) * CHUNK],
                            osb[:])

    nc.compile()
    return nc


def _get_nc():
    if "nc" not in _CACHE:
        _CACHE["nc"] = _build_nc()
    return _CACHE["nc"]


def kernel(query, key, value, Wq, bq, Wk, bk, Wv, bv, Wo, bo):
    from concourse.bass_utils import run_bass_kernel_spmd

    bf16 = ml_dtypes.bfloat16
    query = np.asarray(query, np.float32)
    key = np.asarray(key, np.float32)
    value = np.asarray(value, np.float32)
    Wq, bq = np.asarray(Wq, np.float32), np.asarray(bq, np.float32)
    Wk = np.asarray(Wk, np.float32)
    Wv, bv = np.asarray(Wv, np.float32), np.asarray(bv, np.float32)
    Wo, bo = np.asarray(Wo, np.float32), np.asarray(bo, np.float32)

    nc = _get_nc()

    shared = {
        "wqT": np.ascontiguousarray(Wq.T).astype(bf16),
        "wkT": np.ascontiguousarray(Wk.T).astype(bf16),
        "wvT": np.ascontiguousarray(Wv.T).astype(bf16),
        "woT": np.ascontiguousarray(Wo.T).astype(bf16),
        "bqs": np.ascontiguousarray((bq / 8.0).reshape(DTILES, P).T),
        "bob": np.ascontiguousarray(
            np.broadcast_to(bo + Wo @ bv, (P, D))).astype(np.float32),
    }
    xkTs = [np.ascontiguousarray(key[b].T).astype(bf16) for b in range(B)]
    xvTs = [np.ascontiguousarray(value[b].T).astype(bf16) for b in range(B)]

    in_maps = []
    for c in range(NCORES):
        b, half = divmod(c, 2)
        xq = query[b, half * SQ:(half + 1) * SQ, :]
        in_maps.append({
            **shared,
            "xqT": np.ascontiguousarray(xq.T).astype(bf16),
            "xkT": xkTs[b],
            "xvT": xvTs[b],
        })

    res = run_bass_kernel_spmd(nc, in_maps, list(range(NCORES)))

    outp = np.empty((B, S, D), np.float32)
    for c in range(NCORES):
        b, half = divmod(c, 2)
        outp[b, half * SQ:(half + 1) * SQ, :] = res.results[c]["out"]
    return outp


# revision 8
# speedup vs baseline: 1.8799x; 1.8799x over previous
"""Multi-head attention (B=4, S=2048, D=1024, H=16) on 8 Trainium2 NeuronCores.

Sharding: core c handles batch b = c//2 and query-row half c%2 (1024 query
rows). Each core computes K/V projections for its batch's full 2048 kv rows
(duplicated across the pair), attention for its 1024 query rows over all 16
heads, and the output projection for its rows. Output is a pure concatenation
across cores — no collectives.

Device algorithm (per core, all matmuls bf16 with fp32 PSUM accumulation):
  QT[e,s]  = (WqT.T @ xqT) * 1/8 + bq/8          (e on partitions)
  KT[e,s]  =  WkT.T @ xkT                        (bk dropped: softmax-invariant)
  V[s,e]   =  xvT.T @ WvT                        (bv folded into output bias)
  per head h, q-chunk:
    scoresT[k,q] = KT_h.T @ QT_h                 (K=64 contraction; head parity
                                                  lands on partition halves ->
                                                  row-group-packed matmuls)
    E = exp(scoresT)            on ScalarE, PSUM->SBUF bf16
    [y_h; rowsum] = [V_h | 1].T @ E              (M=65, ones col gives rowsum)
    yT_h = y_h * (1/rowsum)     (PE K=1 matmul replicates 1/rowsum across
                                 partitions; DVE multiply)
  out[q,e] = yT.T @ WoT + (bo + Wo@bv)
"""

import numpy as np
import ml_dtypes

B, S, D = 4, 2048, 1024
H, DK = 16, 64
NCORES = 8
SQ = S // 2            # query rows per core
P = 128
DTILES = D // P        # 8 contraction tiles
QCH = SQ // 512        # 2 q chunks of 512
KT_N = S // P          # 16 kv 128-tiles
HP = H // 2            # 8 head pairs
CHUNK = 512

_CACHE = {}


def _build_nc():
    import concourse.mybir as mybir
    import concourse.tile as tile
    from concourse import bacc

    F32, BF16 = mybir.dt.float32, mybir.dt.bfloat16
    Copy = mybir.ActivationFunctionType.Copy
    Exp = mybir.ActivationFunctionType.Exp

    nc = bacc.Bacc("TRN2", target_bir_lowering=False, debug=False,
                   num_devices=NCORES)

    xqT = nc.dram_tensor("xqT", [D, SQ], BF16, kind="ExternalInput").ap()
    xkT = nc.dram_tensor("xkT", [D, S], BF16, kind="ExternalInput").ap()
    xvT = nc.dram_tensor("xvT", [D, S], BF16, kind="ExternalInput").ap()
    wqT = nc.dram_tensor("wqT", [D, D], BF16, kind="ExternalInput").ap()
    wkT = nc.dram_tensor("wkT", [D, D], BF16, kind="ExternalInput").ap()
    wvT = nc.dram_tensor("wvT", [D, D], BF16, kind="ExternalInput").ap()
    woT = nc.dram_tensor("woT", [D, D], BF16, kind="ExternalInput").ap()
    bqs = nc.dram_tensor("bqs", [P, DTILES], F32, kind="ExternalInput").ap()
    bob = nc.dram_tensor("bob", [P, D], F32, kind="ExternalInput").ap()
    out = nc.dram_tensor("out", [SQ, D], F32, kind="ExternalOutput").ap()

    def pdt(ap):  # [D, N] dram -> [P, DTILES, N] partition-tiled view
        return ap.rearrange("(a p) n -> p a n", p=P)

    with tile.TileContext(nc) as tc:
        with (
            tc.tile_pool(name="wpool", bufs=2) as wpool,
            tc.tile_pool(name="xpool", bufs=2) as xpool,
            tc.tile_pool(name="cpool", bufs=1) as cpool,
            tc.tile_pool(name="epool", bufs=5) as epool,
            tc.tile_pool(name="ytpool", bufs=2) as ytpool,
            tc.tile_pool(name="npool", bufs=2) as npool,
            tc.tile_pool(name="opool", bufs=2) as opool,
            tc.tile_pool(name="psP", bufs=2, space="PSUM") as psP,
            tc.tile_pool(name="psS", bufs=2, space="PSUM") as psS,
            tc.tile_pool(name="psA", bufs=2, space="PSUM") as psA,
        ):
            # ---- constants / residents ----
            wq_s = wpool.tile([P, DTILES, D], BF16, tag="w", name="wq_s")
            nc.sync.dma_start(wq_s[:], pdt(wqT))
            bq_s = cpool.tile([P, DTILES], F32, name="bq_s")
            nc.sync.dma_start(bq_s[:], bqs[:])
            bob_s = cpool.tile([P, D], F32, name="bob_s")
            nc.sync.dma_start(bob_s[:], bob[:])
            ones_s = cpool.tile([1, DK], BF16, name="ones_s")
            nc.gpsimd.memset(ones_s[:], 1.0)

            qt_s = cpool.tile([P, DTILES, SQ], BF16, name="qt_s")
            kt_s = cpool.tile([P, DTILES, S], BF16, name="kt_s")
            va_s = cpool.tile([P, KT_N, H * (DK + 1)], BF16, name="va_s")
            nc.gpsimd.memset(va_s[:], 1.0)  # ones cols survive the V copies

            # ---- Q projection: QT = WqT.T @ xqT, scaled by 1/8, +bq/8 ----
            for qc in range(QCH):
                xq_c = xpool.tile([P, DTILES, CHUNK], BF16, tag="x", name="xq_c")
                nc.sync.dma_start(
                    xq_c[:], pdt(xqT[:, qc * CHUNK:(qc + 1) * CHUNK]))
                for et in range(DTILES):
                    psq = psP.tile([P, CHUNK], F32, tag="p", name="psq")
                    for dt in range(DTILES):
                        nc.tensor.matmul(
                            psq[:],
                            wq_s[:, dt, et * P:(et + 1) * P],
                            xq_c[:, dt, :],
                            start=(dt == 0), stop=(dt == DTILES - 1))
                    nc.vector.tensor_scalar(
                        qt_s[:, et, qc * CHUNK:(qc + 1) * CHUNK], psq[:],
                        0.125, bq_s[:, et:et + 1],
                        mybir.AluOpType.mult, mybir.AluOpType.add)

            # ---- K projection: KT = WkT.T @ xkT ----
            wk_s = wpool.tile([P, DTILES, D], BF16, tag="w", name="wk_s")
            nc.sync.dma_start(wk_s[:], pdt(wkT))
            for sc in range(S // CHUNK):
                xk_c = xpool.tile([P, DTILES, CHUNK], BF16, tag="x", name="xk_c")
                nc.sync.dma_start(
                    xk_c[:], pdt(xkT[:, sc * CHUNK:(sc + 1) * CHUNK]))
                for et in range(DTILES):
                    psk = psP.tile([P, CHUNK], F32, tag="p", name="psk")
                    for dt in range(DTILES):
                        nc.tensor.matmul(
                            psk[:],
                            wk_s[:, dt, et * P:(et + 1) * P],
                            xk_c[:, dt, :],
                            start=(dt == 0), stop=(dt == DTILES - 1))
                    nc.vector.tensor_copy(
                        kt_s[:, et, sc * CHUNK:(sc + 1) * CHUNK], psk[:])

            # ---- V projection: V = xvT.T @ WvT (no bias; folded into bob) ----
            wv_s = wpool.tile([P, DTILES, D], BF16, tag="w", name="wv_s")
            nc.sync.dma_start(wv_s[:], pdt(wvT))
            for sc in range(S // CHUNK):
                xv_c = xpool.tile([P, DTILES, CHUNK], BF16, tag="x", name="xv_c")
                nc.sync.dma_start(
                    xv_c[:], pdt(xvT[:, sc * CHUNK:(sc + 1) * CHUNK]))
                for stl in range(CHUNK // P):
                    st = sc * (CHUNK // P) + stl
                    for ec in range(D // CHUNK):
                        psv = psP.tile([P, CHUNK], F32, tag="p", name="psv")
                        for dt in range(DTILES):
                            nc.tensor.matmul(
                                psv[:],
                                xv_c[:, dt, stl * P:(stl + 1) * P],
                                wv_s[:, dt, ec * CHUNK:(ec + 1) * CHUNK],
                                start=(dt == 0), stop=(dt == DTILES - 1))
                        va_v = va_s.rearrange("p k (h c) -> p k h c", c=DK + 1)
                        nh = CHUNK // DK
                        nc.vector.tensor_copy(
                            va_v[:, st, ec * nh:(ec + 1) * nh, 0:DK],
                            psv.rearrange("p (h c) -> p h c", c=DK))

            # ---- load WoT while attention runs ----
            wo_s = wpool.tile([P, DTILES, D], BF16, tag="w", name="wo_s")
            nc.sync.dma_start(wo_s[:], pdt(woT))

            # ---- attention + output projection, per q chunk ----
            for qc in range(QCH):
                qsl = slice(qc * CHUNK, (qc + 1) * CHUNK)
                yt_c = ytpool.tile([P, DTILES, CHUNK], BF16, tag="yt",
                                   name="yt_c")
                pending = []  # deferred (rsb, par, hp) normalize closures

                def flush_normalize():
                    # replicate 1/rowsum across partitions (PE) and scale the
                    # unnormalized yt slice in place (DVE). Deferred one head
                    # pair so the replicate matmul never stalls PE.
                    for rsb_, par_, hp_ in pending:
                        psr = psP.tile([DK, CHUNK], F32, tag="p", name="psr")
                        nc.tensor.matmul(psr[:], ones_s[:], rsb_[:],
                                         start=True, stop=True)
                        ysl = yt_c[DK * par_:DK * (par_ + 1), hp_, :]
                        nc.vector.tensor_mul(ysl, ysl, psr[:])
                    pending.clear()

                for hp in range(HP):
                    # scores + exp, head parity interleaved (row-group
                    # packing); two kt chunks per psum tile -> one [128,1024]
                    # exp ACTIVATE each
                    ex = {}
                    for par in (0, 1):
                        for half in (0, 1):
                            ex[par, half] = epool.tile(
                                [P, KT_N // 2, CHUNK], BF16, tag="e",
                                name="ex")
                    for kt2 in range(KT_N // 2):
                        pst = {}
                        for par in (0, 1):
                            pst[par] = psS.tile([P, 2 * CHUNK], F32, tag="s",
                                                name="pst")
                        for j in (0, 1):
                            kt = 2 * kt2 + j
                            for par in (0, 1):
                                pb = DK * par
                                nc.tensor.matmul(
                                    pst[par][:, j * CHUNK:(j + 1) * CHUNK],
                                    kt_s[pb:pb + DK, hp, kt * P:(kt + 1) * P],
                                    qt_s[pb:pb + DK, hp, qsl],
                                    start=True, stop=True)
                        half, k2h = kt2 // 4, kt2 % 4
                        for par in (0, 1):
                            nc.scalar.activation(
                                ex[par, half][:, 2 * k2h:2 * k2h + 2, :],
                                pst[par][:], Exp)
                    # attn @ [V | 1]; copy unnormalized y + defer normalize
                    new_pending = []
                    for par in (0, 1):
                        h = 2 * hp + par
                        psa = psA.tile([DK + 1, CHUNK], F32, tag="a",
                                       name="psa")
                        for kt in range(KT_N):
                            nc.tensor.matmul(
                                psa[:],
                                va_s[:, kt, h * (DK + 1):(h + 1) * (DK + 1)],
                                ex[par, kt // 8][:, kt % 8, :],
                                start=(kt == 0), stop=(kt == KT_N - 1))
                        nc.vector.tensor_copy(
                            yt_c[DK * par:DK * (par + 1), hp, :],
                            psa[0:DK, :])
                        rcp = npool.tile([1, CHUNK], F32, tag="rcp",
                                         name="rcp")
                        nc.vector.tensor_copy(rcp[:], psa[DK:DK + 1, :])
                        rs = npool.tile([1, CHUNK], F32, tag="rs", name="rs")
                        nc.vector.reciprocal_approx_fast(rs[:], rcp[:])
                        rsb = npool.tile([1, CHUNK], BF16, tag="rsb",
                                         name="rsb", bufs=4)
                        nc.vector.tensor_copy(rsb[:], rs[:])
                        new_pending.append((rsb, par, hp))
                    flush_normalize()
                    pending.extend(new_pending)
                flush_normalize()

                # output projection for this q chunk
                for qtl in range(CHUNK // P):
                    for ec in range(D // CHUNK):
                        psf = psP.tile([P, CHUNK], F32, tag="p", name="psf")
                        for j in range(DTILES):
                            nc.tensor.matmul(
                                psf[:],
                                yt_c[:, j, qtl * P:(qtl + 1) * P],
                                wo_s[:, j, ec * CHUNK:(ec + 1) * CHUNK],
                                start=(j == 0), stop=(j == DTILES - 1))
                        osb = opool.tile([P, CHUNK], F32, tag="o", name="osb")
                        nc.vector.tensor_add(
                            osb[:], psf[:],
                            bob_s[:, ec * CHUNK:(ec + 1) * CHUNK])
                        r0 = qc * CHUNK + qtl * P
                        nc.sync.dma_start(
                            out[r0:r0 + P, ec * CHUNK:(ec + 1) * CHUNK],
                            osb[:])

    nc.compile()
    return nc


def _get_nc():
    if "nc" not in _CACHE:
        _CACHE["nc"] = _build_nc()
    return _CACHE["nc"]


def kernel(query, key, value, Wq, bq, Wk, bk, Wv, bv, Wo, bo):
    from concourse.bass_utils import run_bass_kernel_spmd

    bf16 = ml_dtypes.bfloat16
    query = np.asarray(query, np.float32)
    key = np.asarray(key, np.float32)
    value = np.asarray(value, np.float32)
    Wq, bq = np.asarray(Wq, np.float32), np.asarray(bq, np.float32)
    Wk = np.asarray(Wk, np.float32)
    Wv, bv = np.asarray(Wv, np.float32), np.asarray(bv, np.float32)
    Wo, bo = np.asarray(Wo, np.float32), np.asarray(bo, np.float32)

    nc = _get_nc()

    shared = {
        "wqT": np.ascontiguousarray(Wq.T).astype(bf16),
        "wkT": np.ascontiguousarray(Wk.T).astype(bf16),
        "wvT": np.ascontiguousarray(Wv.T).astype(bf16),
        "woT": np.ascontiguousarray(Wo.T).astype(bf16),
        "bqs": np.ascontiguousarray((bq / 8.0).reshape(DTILES, P).T),
        "bob": np.ascontiguousarray(
            np.broadcast_to(bo + Wo @ bv, (P, D))).astype(np.float32),
    }
    xkTs = [np.ascontiguousarray(key[b].T).astype(bf16) for b in range(B)]
    xvTs = [np.ascontiguousarray(value[b].T).astype(bf16) for b in range(B)]

    in_maps = []
    for c in range(NCORES):
        b, half = divmod(c, 2)
        xq = query[b, half * SQ:(half + 1) * SQ, :]
        in_maps.append({
            **shared,
            "xqT": np.ascontiguousarray(xq.T).astype(bf16),
            "xkT": xkTs[b],
            "xvT": xvTs[b],
        })

    res = run_bass_kernel_spmd(nc, in_maps, list(range(NCORES)))

    outp = np.empty((B, S, D), np.float32)
    for c in range(NCORES):
        b, half = divmod(c, 2)
        outp[b, half * SQ:(half + 1) * SQ, :] = res.results[c]["out"]
    return outp


# revision 9
# speedup vs baseline: 1.9030x; 1.0123x over previous
"""Multi-head attention (B=4, S=2048, D=1024, H=16) on 8 Trainium2 NeuronCores.

Sharding: core c handles batch b = c//2 and query-row half c%2 (1024 query
rows). Each core computes K/V projections for its batch's full 2048 kv rows
(duplicated across the pair), attention for its 1024 query rows over all 16
heads, and the output projection for its rows. Output is a pure concatenation
across cores — no collectives.

Device algorithm (per core, all matmuls bf16 with fp32 PSUM accumulation):
  QT[e,s]  = (WqT.T @ xqT) * 1/8 + bq/8          (e on partitions)
  KT[e,s]  =  WkT.T @ xkT                        (bk dropped: softmax-invariant)
  V[s,e]   =  xvT.T @ WvT                        (bv folded into output bias)
  per head h, q-chunk:
    scoresT[k,q] = KT_h.T @ QT_h                 (K=64 contraction; head parity
                                                  lands on partition halves ->
                                                  row-group-packed matmuls)
    E = exp(scoresT)            on ScalarE, PSUM->SBUF bf16
    [y_h; rowsum] = [V_h | 1].T @ E              (M=65, ones col gives rowsum)
    yT_h = y_h * (1/rowsum)     (PE K=1 matmul replicates 1/rowsum across
                                 partitions; DVE multiply)
  out[q,e] = yT.T @ WoT + (bo + Wo@bv)
"""

import numpy as np
import ml_dtypes

B, S, D = 4, 2048, 1024
H, DK = 16, 64
NCORES = 8
SQ = S // 2            # query rows per core
P = 128
DTILES = D // P        # 8 contraction tiles
QCH = SQ // 512        # 2 q chunks of 512
KT_N = S // P          # 16 kv 128-tiles
HP = H // 2            # 8 head pairs
CHUNK = 512

_CACHE = {}


def _build_nc():
    import concourse.mybir as mybir
    import concourse.tile as tile
    from concourse import bacc

    F32, BF16 = mybir.dt.float32, mybir.dt.bfloat16
    Copy = mybir.ActivationFunctionType.Copy
    Exp = mybir.ActivationFunctionType.Exp

    nc = bacc.Bacc("TRN2", target_bir_lowering=False, debug=False,
                   num_devices=NCORES)

    xqT = nc.dram_tensor("xqT", [D, SQ], BF16, kind="ExternalInput").ap()
    xkT = nc.dram_tensor("xkT", [D, S], BF16, kind="ExternalInput").ap()
    xvT = nc.dram_tensor("xvT", [D, S], BF16, kind="ExternalInput").ap()
    wqT = nc.dram_tensor("wqT", [D, D], BF16, kind="ExternalInput").ap()
    wkT = nc.dram_tensor("wkT", [D, D], BF16, kind="ExternalInput").ap()
    wvT = nc.dram_tensor("wvT", [D, D], BF16, kind="ExternalInput").ap()
    woT = nc.dram_tensor("woT", [D, D], BF16, kind="ExternalInput").ap()
    bqs = nc.dram_tensor("bqs", [P, DTILES], F32, kind="ExternalInput").ap()
    bob = nc.dram_tensor("bob", [P, D], F32, kind="ExternalInput").ap()
    out = nc.dram_tensor("out", [SQ, D], F32, kind="ExternalOutput").ap()

    def pdt(ap):  # [D, N] dram -> [P, DTILES, N] partition-tiled view
        return ap.rearrange("(a p) n -> p a n", p=P)

    with tile.TileContext(nc) as tc:
        with (
            tc.tile_pool(name="wpool", bufs=2) as wpool,
            tc.tile_pool(name="xpool", bufs=2) as xpool,
            tc.tile_pool(name="cpool", bufs=1) as cpool,
            tc.tile_pool(name="epool", bufs=5) as epool,
            tc.tile_pool(name="ytpool", bufs=2) as ytpool,
            tc.tile_pool(name="npool", bufs=2) as npool,
            tc.tile_pool(name="opool", bufs=2) as opool,
            tc.tile_pool(name="psP", bufs=2, space="PSUM") as psP,
            tc.tile_pool(name="psS", bufs=2, space="PSUM") as psS,
            tc.tile_pool(name="psA", bufs=2, space="PSUM") as psA,
        ):
            # ---- constants / residents ----
            wq_s = wpool.tile([P, DTILES, D], BF16, tag="w", name="wq_s")
            for j in range(4):
                nc.sync.dma_start(wq_s[:, 2 * j:2 * j + 2, :],
                                  pdt(wqT)[:, 2 * j:2 * j + 2, :])
            bq_s = cpool.tile([P, DTILES], F32, name="bq_s")
            nc.sync.dma_start(bq_s[:], bqs[:])
            bob_s = cpool.tile([P, D], F32, name="bob_s")
            nc.sync.dma_start(bob_s[:], bob[:])
            ones_s = cpool.tile([1, DK], BF16, name="ones_s")
            nc.gpsimd.memset(ones_s[:], 1.0)

            qt_s = cpool.tile([P, DTILES, SQ], BF16, name="qt_s")
            kt_s = cpool.tile([P, DTILES, S], BF16, name="kt_s")
            va_s = cpool.tile([P, KT_N, H * (DK + 1)], BF16, name="va_s")
            nc.gpsimd.memset(va_s[:], 1.0)  # ones cols survive the V copies

            # ---- Q projection: QT = WqT.T @ xqT, scaled by 1/8, +bq/8 ----
            for qc in range(QCH):
                xq_c = xpool.tile([P, DTILES, CHUNK], BF16, tag="x", name="xq_c")
                nc.sync.dma_start(
                    xq_c[:], pdt(xqT[:, qc * CHUNK:(qc + 1) * CHUNK]))
                for et in range(DTILES):
                    psq = psP.tile([P, CHUNK], F32, tag="p", name="psq")
                    for dt in range(DTILES):
                        nc.tensor.matmul(
                            psq[:],
                            wq_s[:, dt, et * P:(et + 1) * P],
                            xq_c[:, dt, :],
                            start=(dt == 0), stop=(dt == DTILES - 1))
                    nc.vector.tensor_scalar(
                        qt_s[:, et, qc * CHUNK:(qc + 1) * CHUNK], psq[:],
                        0.125, bq_s[:, et:et + 1],
                        mybir.AluOpType.mult, mybir.AluOpType.add)

            # ---- V projection: V = xvT.T @ WvT (no bias; folded into bob) ----
            wv_s = wpool.tile([P, DTILES, D], BF16, tag="w", name="wv_s")
            for j in range(4):
                nc.sync.dma_start(wv_s[:, 2 * j:2 * j + 2, :],
                                  pdt(wvT)[:, 2 * j:2 * j + 2, :])
            for sc in range(S // CHUNK):
                xv_c = xpool.tile([P, DTILES, CHUNK], BF16, tag="x", name="xv_c")
                nc.sync.dma_start(
                    xv_c[:], pdt(xvT[:, sc * CHUNK:(sc + 1) * CHUNK]))
                for stl in range(CHUNK // P):
                    st = sc * (CHUNK // P) + stl
                    for ec in range(D // CHUNK):
                        psv = psP.tile([P, CHUNK], F32, tag="p", name="psv")
                        for dt in range(DTILES):
                            nc.tensor.matmul(
                                psv[:],
                                xv_c[:, dt, stl * P:(stl + 1) * P],
                                wv_s[:, dt, ec * CHUNK:(ec + 1) * CHUNK],
                                start=(dt == 0), stop=(dt == DTILES - 1))
                        va_v = va_s.rearrange("p k (h c) -> p k h c", c=DK + 1)
                        nh = CHUNK // DK
                        nc.vector.tensor_copy(
                            va_v[:, st, ec * nh:(ec + 1) * nh, 0:DK],
                            psv.rearrange("p (h c) -> p h c", c=DK))

            # ---- K projection (head-pair chunks) + attention, interleaved --
            wk_s = wpool.tile([P, DTILES, D], BF16, tag="w", name="wk_s")
            for j in range(4):
                nc.sync.dma_start(wk_s[:, 2 * j:2 * j + 2, :],
                                  pdt(wkT)[:, 2 * j:2 * j + 2, :])
            wo_s = wpool.tile([P, DTILES, D], BF16, tag="w", name="wo_s")

            def kproj_pair(et2):
                # compute kt_s[:, et, :] for et in {2*et2, 2*et2+1}
                for sc in range(S // CHUNK):
                    xk_c = xpool.tile([P, DTILES, CHUNK], BF16, tag="x",
                                      name="xk_c")
                    nc.sync.dma_start(
                        xk_c[:], pdt(xkT[:, sc * CHUNK:(sc + 1) * CHUNK]))
                    for et in (2 * et2, 2 * et2 + 1):
                        psk = psP.tile([P, CHUNK], F32, tag="p", name="psk")
                        for dt in range(DTILES):
                            nc.tensor.matmul(
                                psk[:],
                                wk_s[:, dt, et * P:(et + 1) * P],
                                xk_c[:, dt, :],
                                start=(dt == 0), stop=(dt == DTILES - 1))
                        nc.vector.tensor_copy(
                            kt_s[:, et, sc * CHUNK:(sc + 1) * CHUNK], psk[:])

            pending = []  # deferred normalize closures

            def flush_normalize(yt_c):
                # replicate 1/rowsum across partitions (PE) and scale the
                # unnormalized yt slice in place (DVE). Deferred one head
                # pair so the replicate matmul never stalls PE.
                for rsb_, par_, hp_ in pending:
                    psr = psP.tile([DK, CHUNK], F32, tag="p", name="psr")
                    nc.tensor.matmul(psr[:], ones_s[:], rsb_[:],
                                     start=True, stop=True)
                    ysl = yt_c[DK * par_:DK * (par_ + 1), hp_, :]
                    nc.vector.tensor_mul(ysl, ysl, psr[:])
                pending.clear()

            def attention(hp, qc, yt_c):
                qsl = slice(qc * CHUNK, (qc + 1) * CHUNK)
                # scores + exp, head parity interleaved (row-group packing);
                # two kt chunks per psum tile -> one [128,1024] exp ACTIVATE
                ex = {}
                for par in (0, 1):
                    for half in (0, 1):
                        ex[par, half] = epool.tile(
                            [P, KT_N // 2, CHUNK], BF16, tag="e", name="ex")
                for kt2 in range(KT_N // 2):
                    pst = {}
                    for par in (0, 1):
                        pst[par] = psS.tile([P, 2 * CHUNK], F32, tag="s",
                                            name="pst")
                    for j in (0, 1):
                        kt = 2 * kt2 + j
                        for par in (0, 1):
                            pb = DK * par
                            nc.tensor.matmul(
                                pst[par][:, j * CHUNK:(j + 1) * CHUNK],
                                kt_s[pb:pb + DK, hp, kt * P:(kt + 1) * P],
                                qt_s[pb:pb + DK, hp, qsl],
                                start=True, stop=True)
                    half, k2h = kt2 // 4, kt2 % 4
                    for par in (0, 1):
                        nc.scalar.activation(
                            ex[par, half][:, 2 * k2h:2 * k2h + 2, :],
                            pst[par][:], Exp)
                # attn @ [V | 1]; copy unnormalized y + defer normalize
                new_pending = []
                for par in (0, 1):
                    h = 2 * hp + par
                    psa = psA.tile([DK + 1, CHUNK], F32, tag="a", name="psa")
                    for kt in range(KT_N):
                        nc.tensor.matmul(
                            psa[:],
                            va_s[:, kt, h * (DK + 1):(h + 1) * (DK + 1)],
                            ex[par, kt // 8][:, kt % 8, :],
                            start=(kt == 0), stop=(kt == KT_N - 1))
                    nc.vector.tensor_copy(
                        yt_c[DK * par:DK * (par + 1), hp, :], psa[0:DK, :])
                    rcp = npool.tile([1, CHUNK], F32, tag="rcp", name="rcp")
                    nc.vector.tensor_copy(rcp[:], psa[DK:DK + 1, :])
                    rs = npool.tile([1, CHUNK], F32, tag="rs", name="rs")
                    nc.vector.reciprocal_approx_fast(rs[:], rcp[:])
                    rsb = npool.tile([1, CHUNK], BF16, tag="rsb",
                                     name="rsb", bufs=4)
                    nc.vector.tensor_copy(rsb[:], rs[:])
                    new_pending.append((rsb, par, hp))
                flush_normalize(yt_c)
                pending.extend(new_pending)

            def outproj(qc, yt_c):
                for qtl in range(CHUNK // P):
                    for ec in range(D // CHUNK):
                        psf = psP.tile([P, CHUNK], F32, tag="p", name="psf")
                        for j in range(DTILES):
                            nc.tensor.matmul(
                                psf[:],
                                yt_c[:, j, qtl * P:(qtl + 1) * P],
                                wo_s[:, j, ec * CHUNK:(ec + 1) * CHUNK],
                                start=(j == 0), stop=(j == DTILES - 1))
                        osb = opool.tile([P, CHUNK], F32, tag="o", name="osb")
                        nc.vector.tensor_add(
                            osb[:], psf[:],
                            bob_s[:, ec * CHUNK:(ec + 1) * CHUNK])
                        r0 = qc * CHUNK + qtl * P
                        nc.sync.dma_start(
                            out[r0:r0 + P, ec * CHUNK:(ec + 1) * CHUNK],
                            osb[:])

            # qc0 attention chases K projection, head pair by head pair
            yt0 = ytpool.tile([P, DTILES, CHUNK], BF16, tag="yt", name="yt0")
            for et2 in range(4):
                kproj_pair(et2)
                attention(2 * et2, 0, yt0)
                attention(2 * et2 + 1, 0, yt0)
            flush_normalize(yt0)
            # wo loads during the attention phase
            for j in range(4):
                nc.sync.dma_start(wo_s[:, 2 * j:2 * j + 2, :],
                                  pdt(woT)[:, 2 * j:2 * j + 2, :])
            # qc1 attention is pure ACT-bound; qc0 outproj hides in its slack
            yt1 = ytpool.tile([P, DTILES, CHUNK], BF16, tag="yt", name="yt1")
            for hp in range(HP):
                attention(hp, 1, yt1)
                if hp == 1:
                    outproj(0, yt0)
            flush_normalize(yt1)
            outproj(1, yt1)

    nc.compile()
    return nc


def _get_nc():
    if "nc" not in _CACHE:
        _CACHE["nc"] = _build_nc()
    return _CACHE["nc"]


def kernel(query, key, value, Wq, bq, Wk, bk, Wv, bv, Wo, bo):
    from concourse.bass_utils import run_bass_kernel_spmd

    bf16 = ml_dtypes.bfloat16
    query = np.asarray(query, np.float32)
    key = np.asarray(key, np.float32)
    value = np.asarray(value, np.float32)
    Wq, bq = np.asarray(Wq, np.float32), np.asarray(bq, np.float32)
    Wk = np.asarray(Wk, np.float32)
    Wv, bv = np.asarray(Wv, np.float32), np.asarray(bv, np.float32)
    Wo, bo = np.asarray(Wo, np.float32), np.asarray(bo, np.float32)

    nc = _get_nc()

    shared = {
        "wqT": np.ascontiguousarray(Wq.T).astype(bf16),
        "wkT": np.ascontiguousarray(Wk.T).astype(bf16),
        "wvT": np.ascontiguousarray(Wv.T).astype(bf16),
        "woT": np.ascontiguousarray(Wo.T).astype(bf16),
        "bqs": np.ascontiguousarray((bq / 8.0).reshape(DTILES, P).T),
        "bob": np.ascontiguousarray(
            np.broadcast_to(bo + Wo @ bv, (P, D))).astype(np.float32),
    }
    xkTs = [np.ascontiguousarray(key[b].T).astype(bf16) for b in range(B)]
    xvTs = [np.ascontiguousarray(value[b].T).astype(bf16) for b in range(B)]

    in_maps = []
    for c in range(NCORES):
        b, half = divmod(c, 2)
        xq = query[b, half * SQ:(half + 1) * SQ, :]
        in_maps.append({
            **shared,
            "xqT": np.ascontiguousarray(xq.T).astype(bf16),
            "xkT": xkTs[b],
            "xvT": xvTs[b],
        })

    res = run_bass_kernel_spmd(nc, in_maps, list(range(NCORES)))

    outp = np.empty((B, S, D), np.float32)
    for c in range(NCORES):
        b, half = divmod(c, 2)
        outp[b, half * SQ:(half + 1) * SQ, :] = res.results[c]["out"]
    return outp
